# revision 85
# baseline (speedup 1.0000x reference)
"""BigBird block-sparse attention (3-block sliding window, zero-padded edges)
for Trainium2, SPMD over 8 NeuronCores, data-parallel over the batch dim.

Full computation per batch element b:
  q/k/v = x @ W{q,k,v}        -> [N, H*64]
  block attention: each 128-row query block attends keys of blocks
  {i-1, i, i+1}; out-of-range blocks are zero keys/values that contribute
  exp(0)=1 to the softmax denominator only.
  y = attn_out @ Wo + bo

Matmuls run in bf16 (fp32 accumulation in PSUM).
"""

import os
import numpy as np

import concourse.bass as bass
import concourse.mybir as mybir
import concourse.tile as tile
from concourse import bacc
from concourse.bass_utils import run_bass_kernel_spmd
from concourse.masks import make_identity

B, N, DIM = 16, 1536, 1536
H, DK, DV, BS = 8, 64, 64, 128
NB = N // BS                     # 12 blocks per sequence
NCORES = 8
BPC = B // NCORES                # batch elements per core
KC_ = DIM // 128                 # contraction chunks (d-major x layout)
SCALE = 1.0 / np.sqrt(DK)        # 0.125

f32 = mybir.dt.float32
bf16 = mybir.dt.bfloat16
EXP = mybir.ActivationFunctionType.Exp

_NC_CACHE = {}
LAST_RESULTS = None


def _emit(nc, reps=1, trace_sim=False, phases="PAO", fuse=False,
          psum=(2, 3, 3), evict="dve",
          yb=2, ob=6, y_q="sp", x_q="sp", ot_mode="pe", eb=40, ptp_pool="po", rb=6):
    # x and the big weights arrive pre-cast to bf16 (host-side numpy cast):
    # halves input DMA bytes and removes every on-device cast.  x is also
    # pre-transposed on the host to d-major, tiled per 384-row group and
    # PER-PARTITION contiguous [BPC, 4, 128, KC, 384] (9216B DRAM line per
    # partition -> full DMA rate), so the device needs no transposes.
    # Weights likewise arrive partition-major [128, KC, cols].
    X = nc.dram_tensor("x", [BPC, 4, 128, KC_, 3 * BS], bf16,
                       kind="ExternalInput")
    WQ = nc.dram_tensor("Wq", [128, DIM // 128, H * DK], bf16,
                        kind="ExternalInput")
    WK = nc.dram_tensor("Wk", [128, DIM // 128, H * DK], bf16,
                        kind="ExternalInput")
    WV = nc.dram_tensor("Wv", [128, DIM // 128, H * DV], bf16,
                        kind="ExternalInput")
    WO = nc.dram_tensor("Wo", [128, (H * DV) // 128, DIM], bf16,
                        kind="ExternalInput")
    BO = nc.dram_tensor("bo", [DIM], f32, kind="ExternalInput")
    # y leaves the device as bf16 (host converts to f32): halves the
    # output DMA bytes and speeds the DVE bias-add eviction.
    Y = nc.dram_tensor("y", [BPC, N, DIM], bf16, kind="ExternalOutput")

    KC = DIM // 128              # 12 contraction chunks for projections
    HV = H * DV                  # 512

    with tile.TileContext(nc, trace_sim=trace_sim) as tc:
        with (
            tc.tile_pool(name="wts", bufs=1) as wts,
            tc.tile_pool(name="xtp", bufs=3) as xtp,
            tc.tile_pool(name="qkv", bufs=2) as qkv,
            tc.tile_pool(name="expp", bufs=eb) as expp,
            tc.tile_pool(name="osp", bufs=4) as osp,
            tc.tile_pool(name="otp", bufs=ob) as otp,
            tc.tile_pool(name="rcp", bufs=rb) as rcp,
            tc.tile_pool(name="yp", bufs=yb) as yp,
            tc.tile_pool(name="bigp", bufs=psum[0], space="PSUM") as bigp,
            tc.tile_pool(name="scp", bufs=psum[1], space="PSUM") as scp,
            tc.tile_pool(name="pop", bufs=psum[2], space="PSUM") as pop,
        ):
            # ---- constants ----
            pad128 = wts.tile([128, 1], f32)
            nc.vector.memset(pad128, 128.0)
            ident_bf = wts.tile([128, 128], bf16)
            make_identity(nc, ident_bf)
            bo_bc = wts.tile([128, DIM], f32)

            def load_bo():
                # emitted after the first x issues: the 768KB broadcast
                # rides the scalar ring BEHIND x0 (ring FIFO), out of the
                # startup-critical window; plain RAW deps, no gating.
                bo_ap = BO[:]
                nc.scalar.dma_start(
                    out=bo_bc,
                    in_=bass.AP(tensor=bo_ap.tensor, offset=bo_ap.offset,
                                ap=[[0, 128]] + list(bo_ap.ap)),
                )


            # ---- weights: direct bf16 DMA in consumption order Wq, Wk,
            # Wv, Wo (512-col chunks so Wq completes early and the first
            # projections can start ~7us in) ----
            wq_bf = wts.tile([128, KC, HV], bf16)
            wk_bf = wts.tile([128, KC, HV], bf16)
            wv_bf = wts.tile([128, KC, HV], bf16)
            wo_bf = wts.tile([128, HV // 128, DIM], bf16)
            # Wq in two chunks so the first projections can start before
            # the whole matrix lands.  Wk/Wv/Wo are DMAed from inside the
            # body, gated behind dummy WAR reads, so they don't dilute DMA
            # bandwidth away from the startup-critical x0+Wq transfers
            # (the DMA engines fair-share across in-flight descriptors).
            nc.sync.dma_start(out=wq_bf[:, 0:KC // 2], in_=WQ[:, 0:KC // 2])
            nc.sync.dma_start(out=wq_bf[:, KC // 2:], in_=WQ[:, KC // 2:])
            gate = wts.tile([1, 1], bf16)
            for wsb in (wk_bf, wv_bf, wo_bf):
                nc.vector.memset(wsb[0:1, 0, 0:1], 0.0)

            def load_w(wdram, wsb, with_bo=False):
                nc.vector.tensor_copy(gate, wsb[0:1, 0, 0:1])
                nc.sync.dma_start(out=wsb, in_=wdram[:])

            def mk_state(b):
                if True:
                    # per-batch-element activation buffers (transposed layouts)
                    qT_buf = qkv.tile([128, 4, N], bf16, name=f"qT{b}", tag="qT")
                    kT_buf = qkv.tile([128, 4, N], bf16, name=f"kT{b}", tag="kT")
                    v_buf = qkv.tile([128, NB, H, DV + 1], bf16, name=f"v{b}", tag="v")
                    nc.vector.memset(v_buf[:, :, :, DV:DV + 1], 1.0)

                    # ---- fused pipeline: projections (256-row pairs)
                    # interleaved with scores/exp, attention-out, and output
                    # projection as soon as dependencies allow ----
                    # expt[(hp, s)][j] = exp tile for head 2hp+s, key block j
                    expt = {(hp, s): [None] * NB
                            for hp in range(H // 2) for s in range(2)}

                    def do_x(i3, b=b, split=False, gated=False):
                        # x arrives pre-transposed (d-major): one DMA per
                        # 384-row group, on the scalar queue so it doesn't
                        # sit behind the weight stream on the sync ring.
                        # split: two half transfers (2x fair-share of DMA
                        # bandwidth at startup + kc 0-5 usable early).
                        # gated: dummy WAR read delays the transfer until
                        # this point in the DVE stream is reached.
                        # xT[p, kc, r] = x[b, r0 + r, kc*128 + p]
                        xT = xtp.tile([128, KC, 3 * BS], bf16,
                                      name=f"xT{b}_{i3}", tag="xT")
                        if gated:
                            nc.vector.tensor_copy(gate, xT[0:1, 0, 0:1])
                        if split:
                            h = KC // 2
                            nc.scalar.dma_start(out=xT[:, 0:h], in_=X[b, i3, :, 0:h])
                            nc.scalar.dma_start(out=xT[:, h:], in_=X[b, i3, :, h:])
                        else:
                            nc.scalar.dma_start(out=xT, in_=X[b, i3])
                        return xT

                    ev = nc.vector.tensor_copy if evict == "dve" else nc.scalar.copy

                    def do_QK(i3, xT, b=b, hook=None):
                        # qT / kT: one psum tile per hv-chunk, N=384 per matmul
                        r0 = i3 * 3 * BS
                        for (wsb, obuf) in ((wq_bf, qT_buf), (wk_bf, kT_buf)):
                            for c in range(4):
                                pp = bigp.tile([128, 3 * BS], f32,
                                               name=f"pp{b}_{i3}_{c}", tag="big")
                                for kc in range(KC):
                                    nc.tensor.matmul(
                                        pp,
                                        wsb[:, kc, c * 128:(c + 1) * 128],
                                        xT[:, kc, :],
                                        start=(kc == 0),
                                        stop=(kc == KC - 1),
                                    )
                                ev(obuf[:, c, r0:r0 + 3 * BS], pp)
                                if hook is not None:
                                    hook()
                                    hook = None

                    def do_V(i3, xT, b=b):
                        for t in range(3):
                            pv = bigp.tile([128, HV], f32,
                                           name=f"pv{b}_{i3}_{t}", tag="big")
                            for kc in range(KC):
                                nc.tensor.matmul(
                                    pv, xT[:, kc, t * BS:(t + 1) * BS],
                                    wv_bf[:, kc, :],
                                    start=(kc == 0), stop=(kc == KC - 1))
                            ev(v_buf[:, i3 * 3 + t, :, 0:DV],
                               pv.rearrange("p (h d) -> p h d", h=H))

                    def do_scores(hp, s, j, b=b, expt=expt):
                        qlo, qhi = max(j - 1, 0), min(j + 1, NB - 1)
                        nq = (qhi - qlo + 1) * BS
                        pb = s * DK
                        psc = scp.tile([128, 3 * BS], f32,
                                       name=f"psc{b}_{hp}_{j}_{s}", tag="sc")
                        nc.tensor.matmul(
                            psc[:, 0:nq],
                            kT_buf[pb:pb + DK, hp, j * BS:(j + 1) * BS],
                            qT_buf[pb:pb + DK, hp, qlo * BS:(qhi + 1) * BS],
                            start=True, stop=True,
                        )
                        et = expp.tile([128, 3 * BS], bf16,
                                       name=f"et{b}_{hp}_{j}_{s}", tag="exp")
                        nc.scalar.activation(out=et[:, 0:nq], in_=psc[:, 0:nq],
                                             func=EXP, scale=float(SCALE))
                        expt[(hp, s)][j] = et

                    def do_attn_out(i, hp, osb_i, b=b, expt=expt):
                        # out[q, dv] for heads 2hp, 2hp+1; sums in col DV
                        po = pop.tile([128, 2, DV + 8], f32,
                                      name=f"po{b}_{hp}_{i}", tag="po")
                        js = [j for j in (i - 1, i, i + 1) if 0 <= j < NB]
                        nmm = len(js) * 2
                        idx = 0
                        for j in js:
                            col = (i - max(j - 1, 0)) * BS
                            for s in range(2):
                                nc.tensor.matmul(
                                    po[:, s, 0:DV + 1],
                                    expt[(hp, s)][j][:, col:col + BS],
                                    v_buf[:, j, hp * 2 + s, :],
                                    start=(idx == 0),
                                    stop=(idx == nmm - 1),
                                )
                                idx += 1
                        if i == 0 or i == NB - 1:
                            # zero-padded edge block: 128 keys with logit 0
                            nc.scalar.activation(
                                out=po[:, 0:2, DV:DV + 1],
                                in_=po[:, 0:2, DV:DV + 1],
                                func=mybir.ActivationFunctionType.Identity,
                                bias=pad128, scale=1.0)
                        rc = rcp.tile([128, 2, 1], f32,
                                      name=f"rc{b}_{hp}_{i}", tag="rc")
                        nc.vector.reciprocal(rc, po[:, 0:2, DV:DV + 1])
                        # softmax division: ONE DVE multiply for both heads
                        # of the pair, broadcasting rc along dv via a
                        # stride-0 free dim (halves per-op overhead vs two
                        # ops, and keeps the overloaded scalar engine free)
                        rc_ap = rc[:, 0:2, 0:1]
                        rc_bc = bass.AP(
                            tensor=rc_ap.tensor, offset=rc_ap.offset,
                            ap=list(rc_ap.ap)[:-1] + [[0, DV]])
                        nc.vector.tensor_mul(
                            osb_i[:, hp * 2 * DV:(hp * 2 + 2) * DV].rearrange(
                                "p (s d) -> p s d", s=2),
                            po[:, 0:2, 0:DV], rc_bc)

                    def do_O(i, oT, b=b):
                        # output projection for block i (all heads complete)
                        ysb = yp.tile([128, DIM], bf16, name=f"y{b}_{i}", tag="y")
                        for n in range(3):
                            py = bigp.tile([128, 512], f32,
                                           name=f"py{b}_{i}_{n}", tag="big")
                            for c in range(4):
                                nc.tensor.matmul(py, oT[:, c, :],
                                                 wo_bf[:, c, n * 512:(n + 1) * 512],
                                                 start=(c == 0), stop=(c == 3))
                            nc.vector.tensor_add(
                                ysb[:, n * 512:(n + 1) * 512], py,
                                bo_bc[:, n * 512:(n + 1) * 512])
                            yeng = nc.sync if y_q == "sp" else nc.scalar
                            yeng.dma_start(
                                out=Y[b, i * BS:(i + 1) * BS,
                                      n * 512:(n + 1) * 512],
                                in_=ysb[:, n * 512:(n + 1) * 512])

                    cur = {"j": 0, "i": 0}

                    def drain(hi, b=b):
                        # emit scores/attention/output whose deps are ready;
                        # scores(j) needs qT blocks up to min(j+1, NB-1),
                        # attn(i) needs exp blocks up to min(i+1, NB-1)
                        while True:
                            acted = False
                            if cur["j"] < NB and min(cur["j"] + 1, NB - 1) <= hi:
                                for hp in range(H // 2):
                                    for s in range(2):
                                        do_scores(hp, s, cur["j"])
                                cur["j"] += 1
                                acted = True
                            while (cur["i"] < NB
                                   and (cur["j"] == NB
                                        or min(cur["i"] + 1, NB - 1)
                                        <= cur["j"] - 2)):
                                if cur["j"] == NB and cur["i"] == NB - 2:
                                    # final pair: both attentions first, then
                                    # both O-chains (hides division latency)
                                    osbs = {}
                                    for i_f in (NB - 1, NB - 2):
                                        osb_f = osp.tile(
                                            [128, HV], bf16,
                                            name=f"o{b}_{i_f}", tag="osb")
                                        for hp in range(H // 2):
                                            do_attn_out(i_f, hp, osb_f)
                                        osbs[i_f] = osb_f
                                    if "O" in phases:
                                        for i_f in (NB - 1, NB - 2):
                                            oT = otp.tile(
                                                [128, 4, BS], bf16,
                                                name=f"oT{b}_{i_f}", tag="oT")
                                            for c4 in range(4):
                                                ptp = pop.tile(
                                                    [128, BS], bf16,
                                                    name=f"ptp{b}_{i_f}_{c4}",
                                                    tag=ptp_pool)
                                                nc.tensor.transpose(
                                                    ptp,
                                                    osbs[i_f][:, c4 * BS:
                                                              (c4 + 1) * BS],
                                                    ident_bf)
                                                nc.vector.tensor_copy(
                                                    oT[:, c4, :], ptp)
                                            do_O(i_f, oT)
                                    cur["i"] = NB
                                    acted = True
                                    break
                                i_cur = cur["i"]
                                osb_i = osp.tile([128, HV], bf16,
                                                 name=f"o{b}_{i_cur}", tag="osb")
                                for hp in range(H // 2):
                                    do_attn_out(i_cur, hp, osb_i)
                                if "O" in phases:
                                    oT = otp.tile([128, 4, BS], bf16,
                                                  name=f"oT{b}_{i_cur}", tag="oT")
                                    if ot_mode == "dma":
                                        nc.sync.dma_start_transpose(oT, osb_i)
                                    else:
                                        for c4 in range(4):
                                            _pp = pop if ptp_pool == "po" else scp
                                            ptp = _pp.tile([128, BS], bf16,
                                                           name=f"ptp{b}_{i_cur}_{c4}",
                                                           tag=ptp_pool)
                                            nc.tensor.transpose(
                                                ptp, osb_i[:, c4 * BS:(c4 + 1) * BS],
                                                ident_bf)
                                            nc.vector.tensor_copy(oT[:, c4, :], ptp)
                                    do_O(i_cur, oT)
                                cur["i"] += 1
                                acted = True
                            if not acted:
                                return

                    return {"x": do_x, "QK": do_QK, "V": do_V, "drain": drain}

            def body():
                # Two-element software pipeline: element 1's projections are
                # interleaved into element 0's attention drain (buffers are
                # double-buffered), which keeps the PE dense and warm.
                s0 = mk_state(0)
                s1 = mk_state(1)
                xT0 = [None] * 4
                xT1 = [None] * 4
                xT0[0] = s0["x"](0, split=True)
                load_bo()

                def _hook0():
                    load_w(WK, wk_bf)
                    xT0[1] = s0["x"](1, gated=True)

                # P-phase order QK0,QK1,V0,QK2,V1,QK3,V2,V3 — V(i3) trails
                # QK(i3+1) so the Wv chunks (third on the weight ring) land
                # before the first V matmul needs them.
                s0["QK"](0, xT0[0], hook=_hook0)
                xT0[2] = s0["x"](2, gated=True)
                s0["QK"](1, xT0[1], hook=lambda: load_w(WV, wv_bf))
                s0["V"](0, xT0[0])
                load_w(WO, wo_bf)
                xT0[3] = s0["x"](3)
                s0["QK"](2, xT0[2])
                s0["V"](1, xT0[1])
                s0["QK"](3, xT0[3])
                s0["V"](2, xT0[2])
                s0["V"](3, xT0[3])
                if "A" in phases:
                    xT1[0] = s1["x"](0)
                    xT1[1] = s1["x"](1)
                    s1["QK"](0, xT1[0])
                    s0["drain"](2)
                    s1["QK"](1, xT1[1])
                    s0["drain"](4)
                    s1["V"](0, xT1[0])
                    s0["drain"](6)
                    s0["drain"](8)
                    xT1[2] = s1["x"](2)
                    s1["QK"](2, xT1[2])
                    s1["V"](1, xT1[1])
                    s0["drain"](10)
                    s1["V"](2, xT1[2])
                    xT1[3] = s1["x"](3)
                    s1["QK"](3, xT1[3])
                    s0["drain"](NB - 1)
                    s1["V"](3, xT1[3])
                    s1["drain"](NB - 1)
                else:
                    xT1[0] = s1["x"](0)
                    xT1[1] = s1["x"](1)
                    s1["QK"](0, xT1[0])
                    xT1[2] = s1["x"](2)
                    s1["QK"](1, xT1[1])
                    s1["V"](0, xT1[0])
                    xT1[3] = s1["x"](3)
                    s1["QK"](2, xT1[2])
                    s1["V"](1, xT1[1])
                    s1["QK"](3, xT1[3])
                    s1["V"](2, xT1[2])
                    s1["V"](3, xT1[3])

            if reps > 1:
                with tc.For_i(0, reps, 1):
                    body()
            else:
                body()


def _get_nc():
    if "nc" not in _NC_CACHE:
        nc = bacc.Bacc("TRN2", target_bir_lowering=False, debug=False)
        _emit(nc)
        nc.finalize()
        _NC_CACHE["nc"] = nc
    return _NC_CACHE["nc"]


def make_in_maps(x, Wq, Wk, Wv, Wo, bo):
    import ml_dtypes
    b16 = ml_dtypes.bfloat16
    # host-side cast to bf16 + transpose to d-major, tiled per 384-row
    # group, per-partition contiguous: [B, 4, 128, KC, 384].  Weights
    # partition-major [128, KC, cols].
    x = np.ascontiguousarray(
        np.asarray(x).astype(b16).reshape(B, 4, 3 * BS, KC_, 128)
        .transpose(0, 1, 4, 3, 2))

    def wprep(w):
        w = np.asarray(w).astype(b16)
        return np.ascontiguousarray(
            w.reshape(w.shape[0] // 128, 128, w.shape[1]).transpose(1, 0, 2))

    Wq, Wk, Wv, Wo = wprep(Wq), wprep(Wk), wprep(Wv), wprep(Wo)
    bo = np.ascontiguousarray(np.asarray(bo, dtype=np.float32))
    return [
        {"x": x[c * BPC:(c + 1) * BPC], "Wq": Wq, "Wk": Wk, "Wv": Wv,
         "Wo": Wo, "bo": bo}
        for c in range(NCORES)
    ]


def kernel(x, Wq, Wk, Wv, Wo, bo):
    global LAST_RESULTS
    nc = _get_nc()
    in_maps = make_in_maps(x, Wq, Wk, Wv, Wo, bo)
    trace = bool(int(os.environ.get("KERNEL_TRACE", "0")))
    res = run_bass_kernel_spmd(nc, in_maps, list(range(NCORES)), trace=trace)
    LAST_RESULTS = res
    return np.concatenate(
        [res.results[c]["y"] for c in range(NCORES)], axis=0
    ).astype(np.float32)

